# revision 23
# baseline (speedup 1.0000x reference)
"""Multi-head causal self-attention (B=2, L=2048, D=1024, H=16) on 8 TRN2
NeuronCores — bf16 scores + fp8 DoubleRow PV edition.

Sharding: core c handles batch b = c // 4 and head group g = c % 4 (4 heads =
a 256-wide slice of the QKV output dim and the matching columns of Wo).  Each
core computes a full (L, D) f32 partial of the output projection; the host
sums the 4 partials per batch and adds bo.

Speed/precision design (measured on HW, not the cost model: fp8 DoubleRow
runs 1.0 cycles/column — 2x bf16 per unit contraction, not 4x; K=64 matmuls
clock-throttle 1.8x; ldweights hide behind >=213ns matmuls):
- Host pre-transposes/pre-casts everything (weights scaled by 32 so W fp8/
  bf16 behaves); zero on-chip transposes.
- Projections and scores run in bf16 (score noise is the error budget's
  dominant term; fp8 q/k fails the 2e-2 gate).  Scores are causal-trimmed.
- p = exp(s) lands in fp8e5m2 (e4m3 overflows: max causal score is 8.69 and
  exp bias -1 keeps p <= e^7.7), v in fp8e4m3, and the PV contraction runs
  as fp8 DoubleRow (2x bf16).  The fp8 noise averages out over the softmax
  window; rows q < 128 (tiny windows) are recomputed with f16 p/v and
  overwrite OT before the bf16 output projection.
- The Scalar-engine exp (~1 el/cycle/partition, ~90us) and the PE stream
  (~129us) are co-critical; most projections are DEFERRED and woven into
  the ACT-bound attention stalls one tile-group per score-pair, so the PE
  never waits for phase A.  PV lags one pair behind exp (carry), normalize
  lags one head (pending), outproj(qt-1) weaves into qt.

Layouts (per core, partition dim first):
  x16    [128, 8, 2048]   x^T bf16, d-chunked; DMA'd in qt-sized pieces
  w16*   [128, 8, 256]    W^T bf16, values 32*W
  QT16F  per qt: [128, 2, 512] bf16 q^T, partitions = dq of head pair
  KTz16F [128, 16, 4, 128] bf16 k^T per head, zero-padded: head h lives on
                           its 64 partitions so the pair-layout score matmul
                           (K=128 full partitions, full clock) isolates it.
  Vp8    [128, 8, 4, 2, 128] fp8 v in cols 64:128 (so o_un lands on psum
                           partitions 64:128, a legal 64-partition base),
                           ones col 0 (denominator lands on psum partition 0
                           where the custom-DVE reciprocal reads it — it
                           ignores nonzero partition offsets), zeros between.
  OT per qt: [128, 2, 512] normalized attention out^T (32*o), bf16
  Wo16   [128, 2, 1024]    Wo^T / 32, bf16
Normalize: den = pso[0], reciprocal_approx_fast (DVE, reads PSUM) -> gpsimd
partition_broadcast -> one DVE multiply into OT.  The raw-ISA broadcast is
invisible to Tile's dependency tracker, so the three ops are chained via
tc.chain_iter_dep.
"""

import sys

for _p in ("/opt/trn_rl_repo", "/root/.axon_site/_ro/trn_rl_repo"):
    if _p not in sys.path:
        sys.path.append(_p)

from contextlib import ExitStack

import ml_dtypes
import numpy as np

import concourse.bass as bass
import concourse.tile as tile
from concourse import bacc, mybir
from concourse.bass_utils import run_bass_kernel_spmd

F32 = mybir.dt.float32
F32R = mybir.dt.float32r
F16 = mybir.dt.float16
BF16 = mybir.dt.bfloat16
F8 = mybir.dt.float8e4
F8P = mybir.dt.float8e5
DR = mybir.MatmulPerfMode.DoubleRow

E4 = ml_dtypes.float8_e4m3
BF = ml_dtypes.bfloat16

B, L, D, H = 2, 2048, 1024, 16
DK = D // H  # 64
NCORES = 8
GH = 4  # heads per core
C = GH * DK  # 256
QT_TILES = L // 512  # 4
WS = 32.0  # host-side weight scale (power of two)
EXP_SCALE = 0.125 / (WS * WS)  # 2^-13, exact in f32
VP = 128  # V stationary: ones col 0 (denominator), pad, v at 64:128


def _build_program():
    nc = bacc.Bacc("TRN2", target_bir_lowering=False, debug=False, num_devices=NCORES)

    x16_d = nc.dram_tensor("x16", [128, 8, L], BF16, kind="ExternalInput").ap()
    w16_ds = {}
    for nm in ("q", "k", "v"):
        w16_ds[nm] = nc.dram_tensor(f"w16{nm}", [128, 8, C], BF16, kind="ExternalInput").ap()
    wo16_d = nc.dram_tensor("wo16", [128, 2, D], BF16, kind="ExternalInput").ap()
    bq_d = nc.dram_tensor("bq", [C], F32, kind="ExternalInput").ap()
    bk_d = nc.dram_tensor("bk", [C], F32, kind="ExternalInput").ap()
    bv_d = nc.dram_tensor("bv", [C], F32, kind="ExternalInput").ap()
    out_d = nc.dram_tensor("out", [L, D], F32, kind="ExternalOutput").ap()

    with tile.TileContext(nc) as tc, ExitStack() as ctx:
        pool = ctx.enter_context(tc.tile_pool(name="persist", bufs=1))
        cp = ctx.enter_context(tc.tile_pool(name="copies", bufs=3))
        # PSUM: pss 2x[128,2,512] (4 banks) + pso 2x[128,512] (2) + ps 2x[128,512] (2)
        pp2 = ctx.enter_context(tc.tile_pool(name="pp2", bufs=2, space="PSUM"))
        ppo = ctx.enter_context(tc.tile_pool(name="ppo", bufs=2, space="PSUM"))
        scratch = ctx.enter_context(tc.tile_pool(name="scratch", bufs=2, space="PSUM"))

        # ---- persistent tiles ----
        ones_f32 = pool.tile([1, 128], F32)
        nc.gpsimd.memset(ones_f32[:], 1.0)
        ones_r = pool.tile([1, 128], F32R)
        nc.vector.tensor_copy(ones_r[:], ones_f32[:])
        ebias = pool.tile([128, 1], F32)
        nc.gpsimd.memset(ebias[:], -1.0)

        bq_sb = pool.tile([128, 2], F32)
        nc.sync.dma_start(bq_sb[:], bq_d.rearrange("(c p) -> p c", p=128))
        bk_sb = pool.tile([128, 2], F32)
        nc.sync.dma_start(bk_sb[:], bk_d.rearrange("(c p) -> p c", p=128))
        bv_sb = pool.tile([1, C], F32)
        nc.sync.dma_start(bv_sb[:], bv_d[None, :])
        bv_r = pool.tile([1, C], F32R)
        nc.vector.tensor_copy(bv_r[:], bv_sb[:])

        QT16F = [pool.tile([128, 2, 512], BF16, name=f"QT16F_{q}") for q in range(4)]
        KTz16F = pool.tile([128, 16, 4, 128], BF16)
        Vp8 = pool.tile([128, 8, 4, 2, VP], F8)
        OTs = [pool.tile([128, 2, 512], BF16, name=f"OT_{q}") for q in range(4)]
        Wo16 = pool.tile([128, 2, D], BF16)
        nc.sync.dma_start(Wo16[:], wo16_d)
        Vp16 = pool.tile([128, 4, 128], F16)

        # zero/one fills (gpsimd), ordered so early consumers unblock first
        for qt in range(2):
            nc.gpsimd.memset(KTz16F[:, 4 * qt : 4 * qt + 4], 0.0)
        nc.gpsimd.memset(Vp8[:], 1.0)
        for qt in range(2, 4):
            nc.gpsimd.memset(KTz16F[:, 4 * qt : 4 * qt + 4], 0.0)
        nc.gpsimd.memset(Vp16[:], 1.0)

        with nc.allow_low_precision(reason="fp8/bf16 matmul pipeline"):
            # ======== phase A: minimal upfront projections; the rest is
            # deferred and woven into the ACT-bound attention stalls ========
            lp = ctx.enter_context(tc.tile_pool(name="loads", bufs=1))
            w16 = {}
            for nm in ("q", "k", "v"):
                w16[nm] = lp.tile([128, 8, C], BF16, name=f"w16{nm}")
            x16 = lp.tile([128, 8, L], BF16)
            # DMA order: what the first projections need comes first, split
            # across DMA queues for parallelism
            for c in range(4):
                nc.sync.dma_start(
                    w16["q"][:, 2 * c : 2 * c + 2], w16_ds["q"][:, 2 * c : 2 * c + 2]
                )
            for c in range(4):
                nc.sync.dma_start(
                    x16[:, 2 * c : 2 * c + 2, 0:512],
                    x16_d[:, 2 * c : 2 * c + 2, 0:512],
                )
            nc.sync.dma_start(w16["k"][:], w16_ds["k"])
            nc.sync.dma_start(w16["v"][:], w16_ds["v"])
            for qc in range(1, 4):
                nc.sync.dma_start(
                    x16[:, :, qc * 512 : (qc + 1) * 512],
                    x16_d[:, :, qc * 512 : (qc + 1) * 512],
                )
            nc.sync.dma_start(Wo16[:], wo16_d)

            def proj_qk(name, j, qt):
                b_sb = bq_sb if name == "q" else bk_sb
                ps = scratch.tile([128, 512], F32, tag="ps")
                for dci in range(8):
                    nc.tensor.matmul(
                        ps[:],
                        lhsT=w16[name][:, dci, j * 128 : (j + 1) * 128],
                        rhs=x16[:, dci, qt * 512 : (qt + 1) * 512],
                        start=(dci == 0),
                        stop=(dci == 7),
                    )
                if name == "q":
                    nc.vector.tensor_tensor(
                        QT16F[qt][:, j, :],
                        ps[:],
                        b_sb[:, j, None].to_broadcast((128, 512)),
                        mybir.AluOpType.add,
                    )
                else:
                    for half in range(2):
                        h = 2 * j + half
                        hp = 64 * half
                        nc.vector.tensor_tensor(
                            KTz16F[hp : hp + 64, 4 * qt : 4 * qt + 4, h, :],
                            ps[hp : hp + 64, :].rearrange("p (a b) -> p a b", a=4),
                            b_sb[hp : hp + 64, j, None, None].to_broadcast(
                                (64, 4, 128)
                            ),
                            mybir.AluOpType.add,
                        )

            def proj_v(kt):
                ps = scratch.tile([128, 512], F32, tag="ps")
                for dci in range(8):
                    nc.tensor.matmul(
                        ps[:, 0:C],
                        lhsT=x16[:, dci, kt * 128 : (kt + 1) * 128],
                        rhs=w16["v"][:, dci, :],
                        start=(dci == 0),
                        stop=False,
                    )
                nc.tensor.matmul(
                    ps[:, 0:C], lhsT=ones_r[:], rhs=bv_r[:], start=False, stop=True
                )
                vst = cp.tile([128, C], F8, tag="vst", bufs=2)
                nc.vector.tensor_copy(vst[:], ps[:, 0:C])
                nc.gpsimd.tensor_copy(
                    Vp8[:, kt // 2, :, kt % 2, 64 : 64 + DK],
                    vst[:].rearrange("p (a b) -> p a b", a=4),
                )
                if kt == 0:
                    nc.vector.tensor_copy(
                        Vp16[:, :, 64 : 64 + DK],
                        ps[:, 0:C].rearrange("p (a b) -> p a b", a=4),
                    )

            # upfront: only what the very first score tile needs
            proj_qk("q", 0, 0)
            proj_qk("k", 0, 0)

            # deferred groups woven into attention: (need_qt, need_h, fn)
            deferred = [(0, 0, (lambda k: lambda: proj_v(k))(kt)) for kt in range(4)]
            deferred += [
                (0, 2, lambda: proj_qk("k", 1, 0)),
                (0, 2, lambda: proj_qk("q", 1, 0)),
            ]
            for qt in range(1, 4):
                deferred.append((qt, 0, (lambda q: lambda: proj_qk("q", 0, q))(qt)))
                deferred.append((qt, 0, (lambda q: lambda: proj_qk("k", 0, q))(qt)))
                deferred.append((qt, 2, (lambda q: lambda: proj_qk("q", 1, q))(qt)))
                deferred.append((qt, 2, (lambda q: lambda: proj_qk("k", 1, q))(qt)))
                for kt in range(4 * qt, 4 * qt + 4):
                    deferred.append((qt, 0, (lambda k: lambda: proj_v(k))(kt)))

            def drain(qt, h, count=None):
                # FIFO order matches need order, so popping early is safe
                if count is not None:
                    for _ in range(count):
                        if not deferred:
                            break
                        deferred.pop(0)[2]()
                while deferred and deferred[0][0:2] <= (qt, h):
                    deferred.pop(0)[2]()

            # ======== phase C: attention + woven output projection ========
            def normalize(h, qt, pso, ncols=512, ot=None):
                hj, hp = h // 2, 64 * (h % 2)
                rcp = cp.tile([1, 512], F32, tag="rcp", bufs=2)
                rec = nc.vector.reciprocal_approx_fast(
                    rcp[:, 0:ncols], pso[0:1, 0:ncols]
                )
                tc.chain_iter_dep("nrm", rec.ins)
                rb = cp.tile([64, 512], F32, tag="rb", bufs=2)
                pb = nc.gpsimd.partition_broadcast(
                    rb[:, 0:ncols], rcp[:, 0:ncols], channels=64
                )
                tc.chain_iter_dep("nrm", pb.ins)
                dst = OTs[qt][hp : hp + 64, hj, 0:ncols] if ot is None else ot
                ml = nc.vector.tensor_tensor(
                    dst,
                    pso[64:128, 0:ncols],
                    rb[:, 0:ncols],
                    mybir.AluOpType.mult,
                )
                tc.chain_iter_dep("nrm", ml.ins)

            def outproj(qt):
                for sub in range(4):
                    q0 = qt * 512 + sub * 128
                    for e in range(2):
                        psy = scratch.tile([128, 512], F32, tag="ps")
                        for cj in range(2):
                            nc.tensor.matmul(
                                psy[:],
                                lhsT=OTs[qt][:, cj, sub * 128 : (sub + 1) * 128],
                                rhs=Wo16[:, cj, e * 512 : (e + 1) * 512],
                                start=(cj == 0),
                                stop=(cj == 1),
                            )
                        y_sb = cp.tile([128, 512], F32, tag="y", bufs=3)
                        nc.vector.tensor_copy(y_sb[:], psy[:])
                        nc.sync.dma_start(
                            out_d[q0 : q0 + 64, e * 512 : (e + 1) * 512], y_sb[0:64]
                        )
                        nc.sync.dma_start(
                            out_d[q0 + 64 : q0 + 128, e * 512 : (e + 1) * 512],
                            y_sb[64:128],
                        )

            def precise_rows():
                # f16 p/v recompute of q rows 0:128, overwrites OT[0][:, :, 0:128]
                for h in range(GH):
                    hj, hp = h // 2, 64 * (h % 2)
                    ps16 = scratch.tile([128, 512], F32, tag="ps")
                    nc.tensor.matmul(
                        ps16[:, 0:128],
                        lhsT=KTz16F[:, 0, h, :],
                        rhs=QT16F[0][:, hj, 0:128],
                        start=True,
                        stop=True,
                    )
                    p16 = cp.tile([128, 128], F16, tag="p16", bufs=2)
                    nc.scalar.activation(
                        p16[:],
                        ps16[:, 0:128],
                        mybir.ActivationFunctionType.Exp,
                        scale=EXP_SCALE,
                        bias=ebias[:],
                    )
                    nc.gpsimd.affine_select(
                        out=p16[:],
                        in_=p16[:],
                        pattern=[[1, 128]],
                        compare_op=mybir.AluOpType.is_ge,
                        fill=0.0,
                        base=0,
                        channel_multiplier=-1,
                    )
                    pso16 = scratch.tile([128, 512], F32, tag="ps")
                    nc.tensor.matmul(
                        pso16[:128, 0:128],
                        lhsT=Vp16[:, h, :],
                        rhs=p16[:],
                        start=True,
                        stop=True,
                    )
                    normalize(
                        h, 0, pso16, ncols=128, ot=OTs[0][hp : hp + 64, hj, 0:128]
                    )

            def pv(p8, m, h, pso, npairs, qt):
                qlo = max(0, (2 * m - 4 * qt) * 128)
                nc.tensor.matmul(
                    pso[:VP, qlo:512],
                    lhsT=Vp8[:, m, h, :, :],
                    rhs=p8[:, :, qlo:512],
                    start=(m == 0),
                    stop=(m == npairs - 1),
                    perf_mode=DR,
                )

            carry = None
            pending = None
            for qt in range(QT_TILES):
                npairs = 2 * (qt + 1)
                for h in range(GH):
                    hj = h // 2
                    drain(qt, h)
                    pso = ppo.tile([VP, 512], F32, tag="pso", name=f"pso{qt}{h}")
                    for m in range(npairs):
                        pss = pp2.tile([128, 2, 512], F32, tag="pss")
                        for i in range(2):
                            kt = 2 * m + i
                            qlo = max(0, (kt - 4 * qt) * 128)
                            nc.tensor.matmul(
                                pss[:, i, qlo:512],
                                lhsT=KTz16F[:, kt, h, :],
                                rhs=QT16F[qt][:, hj, qlo:512],
                                start=True,
                                stop=True,
                            )
                        p8 = cp.tile([128, 2, 512], F8P, tag="p8", bufs=6)
                        nc.scalar.activation(
                            p8[:],
                            pss[:],
                            mybir.ActivationFunctionType.Exp,
                            scale=EXP_SCALE,
                            bias=ebias[:],
                        )
                        d_even = 2 * m - 4 * qt
                        if d_even >= 0:
                            # boundary-only masks; fully-masked cols left of
                            # the pair are excluded via the PV column trim
                            c0 = d_even * 128
                            nc.gpsimd.affine_select(
                                out=p8[:, 0, c0 : c0 + 128],
                                in_=p8[:, 0, c0 : c0 + 128],
                                pattern=[[1, 128]],
                                compare_op=mybir.AluOpType.is_ge,
                                fill=0.0,
                                base=0,
                                channel_multiplier=-1,
                            )
                            nc.gpsimd.affine_select(
                                out=p8[:, 1, c0 : c0 + 256],
                                in_=p8[:, 1, c0 : c0 + 256],
                                pattern=[[1, 256]],
                                compare_op=mybir.AluOpType.is_ge,
                                fill=0.0,
                                base=-128,
                                channel_multiplier=-1,
                            )
                        if carry is not None:
                            pv(**carry)
                            carry = None
                        if m == 0:
                            if pending is not None:
                                normalize(*pending)
                                pending = None
                            if h == 0 and qt > 0:
                                if qt == 1:
                                    precise_rows()
                                outproj(qt - 1)
                        carry = dict(p8=p8, m=m, h=h, pso=pso, npairs=npairs, qt=qt)
                        drain(qt, h, count=2 if qt == 0 else 1)
                    pending = (h, qt, pso)
            pv(**carry)
            normalize(*pending)
            outproj(QT_TILES - 1)

    nc.compile()
    return nc


_NC_CACHE = None


def _get_program():
    global _NC_CACHE
    if _NC_CACHE is None:
        _NC_CACHE = _build_program()
    return _NC_CACHE


def _chunked(mat_t, nch, cols):
    """[D, cols] -> [128, nch, cols] d-chunked layout."""
    return np.ascontiguousarray(mat_t.reshape(nch, 128, cols).transpose(1, 0, 2))


def _make_in_maps(x, Wq, bq, Wk, bk, Wv, bv, Wo, bo):
    f32 = lambda v: np.asarray(v, dtype=np.float32)
    x = f32(x)
    in_maps = []
    x16_by_b = {}
    for b in range(B):
        xt = np.ascontiguousarray(x[b].T).astype(BF)  # [D, L]
        x16_by_b[b] = _chunked(xt, 8, L)
    for core in range(NCORES):
        b, g = divmod(core, 4)
        s = slice(g * C, (g + 1) * C)
        im = {
            "x16": x16_by_b[b],
            "bq": np.ascontiguousarray(WS * f32(bq)[s]),
            "bk": np.ascontiguousarray(WS * f32(bk)[s]),
            "bv": np.ascontiguousarray(WS * f32(bv)[s]),
        }
        for nm, W in (("q", Wq), ("k", Wk), ("v", Wv)):
            wt = np.ascontiguousarray((WS * f32(W)[s, :]).T).astype(BF)  # [D, C]
            im[f"w16{nm}"] = _chunked(wt, 8, C)
        wo_t = np.ascontiguousarray((f32(Wo)[:, s] / WS).T)  # [C, D]
        im["wo16"] = np.ascontiguousarray(
            wo_t.astype(BF).reshape(2, 128, D).transpose(1, 0, 2)
        )
        in_maps.append(im)
    return in_maps


def _run(in_maps, trace=False, **kw):
    nc = _get_program()
    return run_bass_kernel_spmd(nc, in_maps, list(range(NCORES)), trace=trace, **kw)


def kernel(x, Wq, bq, Wk, bk, Wv, bv, Wo, bo, _trace=False, _trace_out=None, _tmpdir=None):
    in_maps = _make_in_maps(x, Wq, bq, Wk, bk, Wv, bv, Wo, bo)
    res = _run(in_maps, trace=_trace, tmpdir=_tmpdir)
    if _trace_out is not None:
        _trace_out.append(res)
    bo = np.asarray(bo, dtype=np.float32)
    out = np.empty((B, L, D), dtype=np.float32)
    for b in range(B):
        acc = res.results[4 * b]["out"].astype(np.float32)
        for g in range(1, 4):
            acc = acc + res.results[4 * b + g]["out"]
        out[b] = acc + bo[None, :]
    return out


# revision 24
# speedup vs baseline: 1.0068x; 1.0068x over previous
"""Multi-head causal self-attention (B=2, L=2048, D=1024, H=16) on 8 TRN2
NeuronCores — bf16 scores + fp8 DoubleRow PV edition.

Sharding: core c handles batch b = c // 4 and head group g = c % 4 (4 heads =
a 256-wide slice of the QKV output dim and the matching columns of Wo).  Each
core computes a full (L, D) f32 partial of the output projection; the host
sums the 4 partials per batch and adds bo.

Speed/precision design (measured on HW, not the cost model: fp8 DoubleRow
runs 1.0 cycles/column — 2x bf16 per unit contraction, not 4x; K=64 matmuls
clock-throttle 1.8x; ldweights hide behind >=213ns matmuls):
- Host pre-transposes/pre-casts everything (weights scaled by 32 so W fp8/
  bf16 behaves); zero on-chip transposes.
- Projections and scores run in bf16 (score noise is the error budget's
  dominant term; fp8 q/k fails the 2e-2 gate).  Scores are causal-trimmed.
- p = exp(s) lands in fp8e5m2 (e4m3 overflows: max causal score is 8.69 and
  exp bias -1 keeps p <= e^7.7), v in fp8e4m3, and the PV contraction runs
  as fp8 DoubleRow (2x bf16).  The fp8 noise averages out over the softmax
  window; rows q < 128 (tiny windows) are recomputed with f16 p/v and
  overwrite OT before the bf16 output projection.
- The Scalar-engine exp (~1 el/cycle/partition, ~90us) and the PE stream
  (~129us) are co-critical; most projections are DEFERRED and woven into
  the ACT-bound attention stalls one tile-group per score-pair, so the PE
  never waits for phase A.  PV lags one pair behind exp (carry), normalize
  lags one head (pending), outproj(qt-1) weaves into qt.

Layouts (per core, partition dim first):
  x16    [128, 8, 2048]   x^T bf16, d-chunked; DMA'd in qt-sized pieces
  w16*   [128, 8, 256]    W^T bf16, values 32*W
  QT16F  per qt: [128, 2, 512] bf16 q^T, partitions = dq of head pair
  KTz16F [128, 16, 4, 128] bf16 k^T per head, zero-padded: head h lives on
                           its 64 partitions so the pair-layout score matmul
                           (K=128 full partitions, full clock) isolates it.
  Vp8    [128, 8, 4, 2, 128] fp8 v in cols 64:128 (so o_un lands on psum
                           partitions 64:128, a legal 64-partition base),
                           ones col 0 (denominator lands on psum partition 0
                           where the custom-DVE reciprocal reads it — it
                           ignores nonzero partition offsets), zeros between.
  OT per qt: [128, 2, 512] normalized attention out^T (32*o), bf16
  Wo16   [128, 2, 1024]    Wo^T / 32, bf16
Normalize: den = pso[0], reciprocal_approx_fast (DVE, reads PSUM) -> gpsimd
partition_broadcast -> one DVE multiply into OT.  The raw-ISA broadcast is
invisible to Tile's dependency tracker, so the three ops are chained via
tc.chain_iter_dep.
"""

import sys

for _p in ("/opt/trn_rl_repo", "/root/.axon_site/_ro/trn_rl_repo"):
    if _p not in sys.path:
        sys.path.append(_p)

from contextlib import ExitStack

import ml_dtypes
import numpy as np

import concourse.bass as bass
import concourse.tile as tile
from concourse import bacc, mybir
from concourse.bass_utils import run_bass_kernel_spmd

F32 = mybir.dt.float32
F32R = mybir.dt.float32r
F16 = mybir.dt.float16
BF16 = mybir.dt.bfloat16
F8 = mybir.dt.float8e4
F8P = mybir.dt.float8e5
DR = mybir.MatmulPerfMode.DoubleRow

E4 = ml_dtypes.float8_e4m3
BF = ml_dtypes.bfloat16

B, L, D, H = 2, 2048, 1024, 16
DK = D // H  # 64
NCORES = 8
GH = 4  # heads per core
C = GH * DK  # 256
QT_TILES = L // 512  # 4
WS = 32.0  # host-side weight scale (power of two)
EXP_SCALE = 0.125 / (WS * WS)  # 2^-13, exact in f32
VP = 128  # V stationary: ones col 0 (denominator), pad, v at 64:128


def _build_program():
    nc = bacc.Bacc("TRN2", target_bir_lowering=False, debug=False, num_devices=NCORES)

    x16_d = nc.dram_tensor("x16", [128, 8, L], BF16, kind="ExternalInput").ap()
    w16_ds = {}
    for nm in ("q", "k", "v"):
        w16_ds[nm] = nc.dram_tensor(f"w16{nm}", [128, 8, C], BF16, kind="ExternalInput").ap()
    wo16_d = nc.dram_tensor("wo16", [128, 2, D], BF16, kind="ExternalInput").ap()
    bq_d = nc.dram_tensor("bq", [C], F32, kind="ExternalInput").ap()
    bk_d = nc.dram_tensor("bk", [C], F32, kind="ExternalInput").ap()
    bv_d = nc.dram_tensor("bv", [C], F32, kind="ExternalInput").ap()
    out_d = nc.dram_tensor("out", [L, D], F32, kind="ExternalOutput").ap()

    with tile.TileContext(nc) as tc, ExitStack() as ctx:
        pool = ctx.enter_context(tc.tile_pool(name="persist", bufs=1))
        cp = ctx.enter_context(tc.tile_pool(name="copies", bufs=3))
        # PSUM: pss 2x[128,2,512] (4 banks) + pso 2x[128,512] (2) + ps 2x[128,512] (2)
        pp2 = ctx.enter_context(tc.tile_pool(name="pp2", bufs=2, space="PSUM"))
        ppo = ctx.enter_context(tc.tile_pool(name="ppo", bufs=2, space="PSUM"))
        scratch = ctx.enter_context(tc.tile_pool(name="scratch", bufs=2, space="PSUM"))

        # ---- persistent tiles ----
        ones_f32 = pool.tile([1, 128], F32)
        nc.gpsimd.memset(ones_f32[:], 1.0)
        ones_r = pool.tile([1, 128], F32R)
        nc.vector.tensor_copy(ones_r[:], ones_f32[:])
        ebias = pool.tile([128, 1], F32)
        nc.gpsimd.memset(ebias[:], -1.0)

        bq_sb = pool.tile([128, 2], F32)
        nc.sync.dma_start(bq_sb[:], bq_d.rearrange("(c p) -> p c", p=128))
        bk_sb = pool.tile([128, 2], F32)
        nc.sync.dma_start(bk_sb[:], bk_d.rearrange("(c p) -> p c", p=128))
        bv_sb = pool.tile([1, C], F32)
        nc.sync.dma_start(bv_sb[:], bv_d[None, :])
        bv_r = pool.tile([1, C], F32R)
        nc.vector.tensor_copy(bv_r[:], bv_sb[:])

        QT16F = [pool.tile([128, 2, 512], BF16, name=f"QT16F_{q}") for q in range(4)]
        KTz16F = pool.tile([128, 16, 4, 128], BF16)
        Vp8 = pool.tile([128, 8, 4, 2, VP], F8)
        OTs = [pool.tile([128, 2, 512], BF16, name=f"OT_{q}") for q in range(4)]
        Wo16 = pool.tile([128, 2, D], BF16)
        nc.sync.dma_start(Wo16[:], wo16_d)
        Vp16 = pool.tile([128, 4, 128], F16)

        # zero/one fills (gpsimd), ordered so early consumers unblock first
        for qt in range(2):
            nc.gpsimd.memset(KTz16F[:, 4 * qt : 4 * qt + 4], 0.0)
        nc.gpsimd.memset(Vp8[:], 1.0)
        for qt in range(2, 4):
            nc.gpsimd.memset(KTz16F[:, 4 * qt : 4 * qt + 4], 0.0)
        nc.gpsimd.memset(Vp16[:], 1.0)

        with nc.allow_low_precision(reason="fp8/bf16 matmul pipeline"):
            # ======== phase A: minimal upfront projections; the rest is
            # deferred and woven into the ACT-bound attention stalls ========
            lp = ctx.enter_context(tc.tile_pool(name="loads", bufs=1))
            w16 = {}
            for nm in ("q", "k", "v"):
                w16[nm] = lp.tile([128, 8, C], BF16, name=f"w16{nm}")
            x16 = lp.tile([128, 8, L], BF16)
            # DMA order: what the first projections need comes first, split
            # across DMA queues for parallelism
            for c in range(4):
                nc.sync.dma_start(
                    w16["q"][:, 2 * c : 2 * c + 2], w16_ds["q"][:, 2 * c : 2 * c + 2]
                )
            for c in range(4):
                nc.sync.dma_start(
                    x16[:, 2 * c : 2 * c + 2, 0:512],
                    x16_d[:, 2 * c : 2 * c + 2, 0:512],
                )
            nc.sync.dma_start(w16["k"][:], w16_ds["k"])
            nc.sync.dma_start(w16["v"][:], w16_ds["v"])
            for qc in range(1, 4):
                nc.sync.dma_start(
                    x16[:, :, qc * 512 : (qc + 1) * 512],
                    x16_d[:, :, qc * 512 : (qc + 1) * 512],
                )
            nc.sync.dma_start(Wo16[:], wo16_d)

            def proj_qk(name, j, qt):
                b_sb = bq_sb if name == "q" else bk_sb
                ps = scratch.tile([128, 512], F32, tag="ps")
                for dci in range(8):
                    nc.tensor.matmul(
                        ps[:],
                        lhsT=w16[name][:, dci, j * 128 : (j + 1) * 128],
                        rhs=x16[:, dci, qt * 512 : (qt + 1) * 512],
                        start=(dci == 0),
                        stop=(dci == 7),
                    )
                if name == "q":
                    nc.vector.tensor_tensor(
                        QT16F[qt][:, j, :],
                        ps[:],
                        b_sb[:, j, None].to_broadcast((128, 512)),
                        mybir.AluOpType.add,
                    )
                else:
                    for half in range(2):
                        h = 2 * j + half
                        hp = 64 * half
                        nc.vector.tensor_tensor(
                            KTz16F[hp : hp + 64, 4 * qt : 4 * qt + 4, h, :],
                            ps[hp : hp + 64, :].rearrange("p (a b) -> p a b", a=4),
                            b_sb[hp : hp + 64, j, None, None].to_broadcast(
                                (64, 4, 128)
                            ),
                            mybir.AluOpType.add,
                        )

            def proj_v(kt):
                ps = scratch.tile([128, 512], F32, tag="ps")
                for dci in range(8):
                    nc.tensor.matmul(
                        ps[:, 0:C],
                        lhsT=x16[:, dci, kt * 128 : (kt + 1) * 128],
                        rhs=w16["v"][:, dci, :],
                        start=(dci == 0),
                        stop=False,
                    )
                nc.tensor.matmul(
                    ps[:, 0:C], lhsT=ones_r[:], rhs=bv_r[:], start=False, stop=True
                )
                vst = cp.tile([128, C], F8, tag="vst", bufs=2)
                nc.vector.tensor_copy(vst[:], ps[:, 0:C])
                nc.gpsimd.tensor_copy(
                    Vp8[:, kt // 2, :, kt % 2, 64 : 64 + DK],
                    vst[:].rearrange("p (a b) -> p a b", a=4),
                )
                if kt == 0:
                    nc.vector.tensor_copy(
                        Vp16[:, :, 64 : 64 + DK],
                        ps[:, 0:C].rearrange("p (a b) -> p a b", a=4),
                    )

            # upfront: only what the very first score tile needs
            proj_qk("q", 0, 0)
            proj_qk("k", 0, 0)

            # deferred groups woven into attention: (need_qt, need_h, fn)
            deferred = [(0, 0, (lambda k: lambda: proj_v(k))(kt)) for kt in range(4)]
            deferred += [
                (0, 2, lambda: proj_qk("k", 1, 0)),
                (0, 2, lambda: proj_qk("q", 1, 0)),
            ]
            for qt in range(1, 4):
                deferred.append((qt, 0, (lambda q: lambda: proj_qk("q", 0, q))(qt)))
                deferred.append((qt, 0, (lambda q: lambda: proj_qk("k", 0, q))(qt)))
                deferred.append((qt, 2, (lambda q: lambda: proj_qk("q", 1, q))(qt)))
                deferred.append((qt, 2, (lambda q: lambda: proj_qk("k", 1, q))(qt)))
                for kt in range(4 * qt, 4 * qt + 4):
                    deferred.append((qt, 0, (lambda k: lambda: proj_v(k))(kt)))

            def drain(qt, h, count=None):
                # FIFO order matches need order, so popping early is safe
                if count is not None:
                    for _ in range(count):
                        if not deferred:
                            break
                        deferred.pop(0)[2]()
                while deferred and deferred[0][0:2] <= (qt, h):
                    deferred.pop(0)[2]()

            # ======== phase C: attention + woven output projection ========
            def normalize(h, qt, pso, ncols=512, ot=None):
                hj, hp = h // 2, 64 * (h % 2)
                rcp = cp.tile([1, 512], F32, tag="rcp", bufs=2)
                rec = nc.vector.reciprocal_approx_fast(
                    rcp[:, 0:ncols], pso[0:1, 0:ncols]
                )
                tc.chain_iter_dep("nrm", rec.ins)
                rb = cp.tile([64, 512], F32, tag="rb", bufs=2)
                pb = nc.gpsimd.partition_broadcast(
                    rb[:, 0:ncols], rcp[:, 0:ncols], channels=64
                )
                tc.chain_iter_dep("nrm", pb.ins)
                dst = OTs[qt][hp : hp + 64, hj, 0:ncols] if ot is None else ot
                ml = nc.vector.tensor_tensor(
                    dst,
                    pso[64:128, 0:ncols],
                    rb[:, 0:ncols],
                    mybir.AluOpType.mult,
                )
                tc.chain_iter_dep("nrm", ml.ins)

            def outproj(qt):
                for sub in range(4):
                    q0 = qt * 512 + sub * 128
                    for e in range(2):
                        psy = scratch.tile([128, 512], F32, tag="ps")
                        for cj in range(2):
                            nc.tensor.matmul(
                                psy[:],
                                lhsT=OTs[qt][:, cj, sub * 128 : (sub + 1) * 128],
                                rhs=Wo16[:, cj, e * 512 : (e + 1) * 512],
                                start=(cj == 0),
                                stop=(cj == 1),
                            )
                        y_sb = cp.tile([128, 512], F32, tag="y", bufs=3)
                        nc.vector.tensor_copy(y_sb[:], psy[:])
                        nc.sync.dma_start(
                            out_d[q0 : q0 + 64, e * 512 : (e + 1) * 512], y_sb[0:64]
                        )
                        nc.sync.dma_start(
                            out_d[q0 + 64 : q0 + 128, e * 512 : (e + 1) * 512],
                            y_sb[64:128],
                        )

            def precise_rows():
                # f16 p/v recompute of q rows 0:128, overwrites OT[0][:, :, 0:128]
                for h in range(GH):
                    hj, hp = h // 2, 64 * (h % 2)
                    ps16 = scratch.tile([128, 512], F32, tag="ps")
                    nc.tensor.matmul(
                        ps16[:, 0:128],
                        lhsT=KTz16F[:, 0, h, :],
                        rhs=QT16F[0][:, hj, 0:128],
                        start=True,
                        stop=True,
                    )
                    p16 = cp.tile([128, 128], F16, tag="p16", bufs=2)
                    nc.scalar.activation(
                        p16[:],
                        ps16[:, 0:128],
                        mybir.ActivationFunctionType.Exp,
                        scale=EXP_SCALE,
                        bias=ebias[:],
                    )
                    nc.gpsimd.affine_select(
                        out=p16[:],
                        in_=p16[:],
                        pattern=[[1, 128]],
                        compare_op=mybir.AluOpType.is_ge,
                        fill=0.0,
                        base=0,
                        channel_multiplier=-1,
                    )
                    pso16 = scratch.tile([128, 512], F32, tag="ps")
                    nc.tensor.matmul(
                        pso16[:128, 0:128],
                        lhsT=Vp16[:, h, :],
                        rhs=p16[:],
                        start=True,
                        stop=True,
                    )
                    normalize(
                        h, 0, pso16, ncols=128, ot=OTs[0][hp : hp + 64, hj, 0:128]
                    )

            def pv(p8, m, h, pso, npairs, qt):
                qlo = max(0, (2 * m - 4 * qt) * 128)
                nc.tensor.matmul(
                    pso[:VP, qlo:512],
                    lhsT=Vp8[:, m, h, :, :],
                    rhs=p8[:, :, qlo:512],
                    start=(m == 0),
                    stop=(m == npairs - 1),
                    perf_mode=DR,
                )

            carry = None
            pending = None
            for qt in range(QT_TILES):
                npairs = 2 * (qt + 1)
                for h in range(GH):
                    hj = h // 2
                    drain(qt, h)
                    pso = ppo.tile([VP, 512], F32, tag="pso", name=f"pso{qt}{h}")
                    for m in range(npairs):
                        pss = pp2.tile([128, 2, 512], F32, tag="pss")
                        for i in range(2):
                            kt = 2 * m + i
                            qlo = max(0, (kt - 4 * qt) * 128)
                            nc.tensor.matmul(
                                pss[:, i, qlo:512],
                                lhsT=KTz16F[:, kt, h, :],
                                rhs=QT16F[qt][:, hj, qlo:512],
                                start=True,
                                stop=True,
                            )
                        p8 = cp.tile([128, 2, 512], F8P, tag="p8", bufs=6)
                        qlo_p = max(0, (2 * m - 4 * qt) * 128)
                        nc.scalar.activation(
                            p8[:, :, qlo_p:512],
                            pss[:, :, qlo_p:512],
                            mybir.ActivationFunctionType.Exp,
                            scale=EXP_SCALE,
                            bias=ebias[:],
                        )
                        d_even = 2 * m - 4 * qt
                        if d_even >= 0:
                            # boundary-only masks; fully-masked cols left of
                            # the pair are excluded via the PV column trim
                            c0 = d_even * 128
                            nc.gpsimd.affine_select(
                                out=p8[:, 0, c0 : c0 + 128],
                                in_=p8[:, 0, c0 : c0 + 128],
                                pattern=[[1, 128]],
                                compare_op=mybir.AluOpType.is_ge,
                                fill=0.0,
                                base=0,
                                channel_multiplier=-1,
                            )
                            nc.gpsimd.affine_select(
                                out=p8[:, 1, c0 : c0 + 256],
                                in_=p8[:, 1, c0 : c0 + 256],
                                pattern=[[1, 256]],
                                compare_op=mybir.AluOpType.is_ge,
                                fill=0.0,
                                base=-128,
                                channel_multiplier=-1,
                            )
                        if carry is not None:
                            pv(**carry)
                            carry = None
                        if m == 0:
                            if pending is not None:
                                normalize(*pending)
                                pending = None
                            if h == 0 and qt > 0:
                                if qt == 1:
                                    precise_rows()
                                outproj(qt - 1)
                        carry = dict(p8=p8, m=m, h=h, pso=pso, npairs=npairs, qt=qt)
                        drain(qt, h, count=2 if qt == 0 else 1)
                    pending = (h, qt, pso)
            pv(**carry)
            normalize(*pending)
            outproj(QT_TILES - 1)

    nc.compile()
    return nc


_NC_CACHE = None


def _get_program():
    global _NC_CACHE
    if _NC_CACHE is None:
        _NC_CACHE = _build_program()
    return _NC_CACHE


def _chunked(mat_t, nch, cols):
    """[D, cols] -> [128, nch, cols] d-chunked layout."""
    return np.ascontiguousarray(mat_t.reshape(nch, 128, cols).transpose(1, 0, 2))


def _make_in_maps(x, Wq, bq, Wk, bk, Wv, bv, Wo, bo):
    f32 = lambda v: np.asarray(v, dtype=np.float32)
    x = f32(x)
    in_maps = []
    x16_by_b = {}
    for b in range(B):
        xt = np.ascontiguousarray(x[b].T).astype(BF)  # [D, L]
        x16_by_b[b] = _chunked(xt, 8, L)
    for core in range(NCORES):
        b, g = divmod(core, 4)
        s = slice(g * C, (g + 1) * C)
        im = {
            "x16": x16_by_b[b],
            "bq": np.ascontiguousarray(WS * f32(bq)[s]),
            "bk": np.ascontiguousarray(WS * f32(bk)[s]),
            "bv": np.ascontiguousarray(WS * f32(bv)[s]),
        }
        for nm, W in (("q", Wq), ("k", Wk), ("v", Wv)):
            wt = np.ascontiguousarray((WS * f32(W)[s, :]).T).astype(BF)  # [D, C]
            im[f"w16{nm}"] = _chunked(wt, 8, C)
        wo_t = np.ascontiguousarray((f32(Wo)[:, s] / WS).T)  # [C, D]
        im["wo16"] = np.ascontiguousarray(
            wo_t.astype(BF).reshape(2, 128, D).transpose(1, 0, 2)
        )
        in_maps.append(im)
    return in_maps


def _run(in_maps, trace=False, **kw):
    nc = _get_program()
    return run_bass_kernel_spmd(nc, in_maps, list(range(NCORES)), trace=trace, **kw)


def kernel(x, Wq, bq, Wk, bk, Wv, bv, Wo, bo, _trace=False, _trace_out=None, _tmpdir=None):
    in_maps = _make_in_maps(x, Wq, bq, Wk, bk, Wv, bv, Wo, bo)
    res = _run(in_maps, trace=_trace, tmpdir=_tmpdir)
    if _trace_out is not None:
        _trace_out.append(res)
    bo = np.asarray(bo, dtype=np.float32)
    out = np.empty((B, L, D), dtype=np.float32)
    for b in range(B):
        acc = res.results[4 * b]["out"].astype(np.float32)
        for g in range(1, 4):
            acc = acc + res.results[4 * b + g]["out"]
        out[b] = acc + bo[None, :]
    return out


# revision 25
# speedup vs baseline: 1.0409x; 1.0339x over previous
"""Multi-head causal self-attention (B=2, L=2048, D=1024, H=16) on 8 TRN2
NeuronCores — bf16 scores + fp8 DoubleRow PV edition.

Sharding: core c handles batch b = c // 4 and head group g = c % 4 (4 heads =
a 256-wide slice of the QKV output dim and the matching columns of Wo).  Each
core computes a full (L, D) f32 partial of the output projection; the host
sums the 4 partials per batch and adds bo.

Speed/precision design (measured on HW, not the cost model: fp8 DoubleRow
runs 1.0 cycles/column — 2x bf16 per unit contraction, not 4x; K=64 matmuls
clock-throttle 1.8x; ldweights hide behind >=213ns matmuls):
- Host pre-transposes/pre-casts everything (weights scaled by 32 so W fp8/
  bf16 behaves); zero on-chip transposes.
- Projections and scores run in bf16 (score noise is the error budget's
  dominant term; fp8 q/k fails the 2e-2 gate).  Scores are causal-trimmed.
- p = exp(s) lands in fp8e5m2 (e4m3 overflows: max causal score is 8.69 and
  exp bias -1 keeps p <= e^7.7), v in fp8e4m3, and the PV contraction runs
  as fp8 DoubleRow (2x bf16).  The fp8 noise averages out over the softmax
  window; rows q < 128 (tiny windows) are recomputed with f16 p/v and
  overwrite OT before the bf16 output projection.
- The Scalar-engine exp (~1 el/cycle/partition, ~90us) and the PE stream
  (~129us) are co-critical; most projections are DEFERRED and woven into
  the ACT-bound attention stalls one tile-group per score-pair, so the PE
  never waits for phase A.  PV lags one pair behind exp (carry), normalize
  lags one head (pending), outproj(qt-1) weaves into qt.

Layouts (per core, partition dim first):
  x16    [128, 8, 2048]   x^T bf16, d-chunked; DMA'd in qt-sized pieces
  w16*   [128, 8, 256]    W^T bf16, values 32*W
  QT16F  per qt: [128, 2, 512] bf16 q^T, partitions = dq of head pair
  KTz16F [128, 16, 4, 128] bf16 k^T per head, zero-padded: head h lives on
                           its 64 partitions so the pair-layout score matmul
                           (K=128 full partitions, full clock) isolates it.
  Vp8    [128, 8, 4, 2, 128] fp8 v in cols 64:128 (so o_un lands on psum
                           partitions 64:128, a legal 64-partition base),
                           ones col 0 (denominator lands on psum partition 0
                           where the custom-DVE reciprocal reads it — it
                           ignores nonzero partition offsets), zeros between.
  OT per qt: [128, 2, 512] normalized attention out^T (32*o), bf16
  Wo16   [128, 2, 1024]    Wo^T / 32, bf16
Normalize: den = pso[0], reciprocal_approx_fast (DVE, reads PSUM) -> gpsimd
partition_broadcast -> one DVE multiply into OT.  The raw-ISA broadcast is
invisible to Tile's dependency tracker, so the three ops are chained via
tc.chain_iter_dep.
"""

import sys

for _p in ("/opt/trn_rl_repo", "/root/.axon_site/_ro/trn_rl_repo"):
    if _p not in sys.path:
        sys.path.append(_p)

from contextlib import ExitStack

import ml_dtypes
import numpy as np

import concourse.bass as bass
import concourse.tile as tile
from concourse import bacc, mybir
from concourse.bass_utils import run_bass_kernel_spmd

F32 = mybir.dt.float32
F32R = mybir.dt.float32r
F16 = mybir.dt.float16
BF16 = mybir.dt.bfloat16
F8 = mybir.dt.float8e4
F8P = mybir.dt.float8e5
DR = mybir.MatmulPerfMode.DoubleRow

E4 = ml_dtypes.float8_e4m3
BF = ml_dtypes.bfloat16

B, L, D, H = 2, 2048, 1024, 16
DK = D // H  # 64
NCORES = 8
GH = 4  # heads per core
C = GH * DK  # 256
QT_TILES = L // 512  # 4
WS = 32.0  # host-side weight scale (power of two)
EXP_SCALE = 0.125 / (WS * WS)  # 2^-13, exact in f32
VP = 128  # V stationary: ones col 0 (denominator), pad, v at 64:128


def _build_program():
    nc = bacc.Bacc("TRN2", target_bir_lowering=False, debug=False, num_devices=NCORES)

    x16_d = nc.dram_tensor("x16", [128, 8, L], BF16, kind="ExternalInput").ap()
    w16_ds = {}
    for nm in ("q", "k", "v"):
        w16_ds[nm] = nc.dram_tensor(f"w16{nm}", [128, 8, C], BF16, kind="ExternalInput").ap()
    wo16_d = nc.dram_tensor("wo16", [128, 2, D], BF16, kind="ExternalInput").ap()
    bq_d = nc.dram_tensor("bq", [C], F32, kind="ExternalInput").ap()
    bk_d = nc.dram_tensor("bk", [C], F32, kind="ExternalInput").ap()
    bv_d = nc.dram_tensor("bv", [C], F32, kind="ExternalInput").ap()
    out_d = nc.dram_tensor("out", [L, D], F32, kind="ExternalOutput").ap()

    with tile.TileContext(nc) as tc, ExitStack() as ctx:
        pool = ctx.enter_context(tc.tile_pool(name="persist", bufs=1))
        cp = ctx.enter_context(tc.tile_pool(name="copies", bufs=3))
        # PSUM: pss 2x[128,2,512] (4 banks) + pso 2x[128,512] (2) + ps 2x[128,512] (2)
        pp2 = ctx.enter_context(tc.tile_pool(name="pp2", bufs=2, space="PSUM"))
        ppo = ctx.enter_context(tc.tile_pool(name="ppo", bufs=2, space="PSUM"))
        scratch = ctx.enter_context(tc.tile_pool(name="scratch", bufs=2, space="PSUM"))

        # ---- persistent tiles ----
        ones_f32 = pool.tile([1, 128], F32)
        nc.gpsimd.memset(ones_f32[:], 1.0)
        ones_r = pool.tile([1, 128], F32R)
        nc.vector.tensor_copy(ones_r[:], ones_f32[:])
        ebias = pool.tile([128, 1], F32)
        nc.gpsimd.memset(ebias[:], -1.0)

        bq_sb = pool.tile([128, 2], F32)
        nc.sync.dma_start(bq_sb[:], bq_d.rearrange("(c p) -> p c", p=128))
        bk_sb = pool.tile([128, 2], F32)
        nc.sync.dma_start(bk_sb[:], bk_d.rearrange("(c p) -> p c", p=128))
        bv_sb = pool.tile([1, C], F32)
        nc.sync.dma_start(bv_sb[:], bv_d[None, :])
        bv_r = pool.tile([1, C], F32R)
        nc.vector.tensor_copy(bv_r[:], bv_sb[:])

        QT16F = [pool.tile([128, 2, 512], BF16, name=f"QT16F_{q}") for q in range(4)]
        KTz16F = pool.tile([128, 16, 4, 128], BF16)
        Vp8 = pool.tile([128, 8, 4, 2, VP], F8)
        OTs = [pool.tile([128, 2, 512], BF16, name=f"OT_{q}") for q in range(4)]
        Wo16 = pool.tile([128, 2, D], BF16)
        nc.sync.dma_start(Wo16[:], wo16_d)
        Vp16 = pool.tile([128, 4, 128], F16)

        # zero/one fills (gpsimd), ordered so early consumers unblock first
        for qt in range(2):
            nc.gpsimd.memset(KTz16F[:, 4 * qt : 4 * qt + 4], 0.0)
        nc.gpsimd.memset(Vp8[:], 1.0)
        for qt in range(2, 4):
            nc.gpsimd.memset(KTz16F[:, 4 * qt : 4 * qt + 4], 0.0)
        nc.gpsimd.memset(Vp16[:], 1.0)

        with nc.allow_low_precision(reason="fp8/bf16 matmul pipeline"):
            # ======== phase A: minimal upfront projections; the rest is
            # deferred and woven into the ACT-bound attention stalls ========
            lp = ctx.enter_context(tc.tile_pool(name="loads", bufs=1))
            w16 = {}
            for nm in ("q", "k", "v"):
                w16[nm] = lp.tile([128, 8, C], BF16, name=f"w16{nm}")
            x16 = lp.tile([128, 8, L], BF16)
            # DMA order: what the first projections need comes first, split
            # across DMA queues for parallelism
            for c in range(4):
                nc.sync.dma_start(
                    w16["q"][:, 2 * c : 2 * c + 2], w16_ds["q"][:, 2 * c : 2 * c + 2]
                )
            for c in range(4):
                nc.sync.dma_start(
                    x16[:, 2 * c : 2 * c + 2, 0:512],
                    x16_d[:, 2 * c : 2 * c + 2, 0:512],
                )
            nc.sync.dma_start(w16["k"][:], w16_ds["k"])
            nc.sync.dma_start(w16["v"][:], w16_ds["v"])
            for qc in range(1, 4):
                nc.sync.dma_start(
                    x16[:, :, qc * 512 : (qc + 1) * 512],
                    x16_d[:, :, qc * 512 : (qc + 1) * 512],
                )
            nc.sync.dma_start(Wo16[:], wo16_d)

            def proj_qk(name, j, qt):
                b_sb = bq_sb if name == "q" else bk_sb
                ps = scratch.tile([128, 512], F32, tag="ps")
                for dci in range(8):
                    nc.tensor.matmul(
                        ps[:],
                        lhsT=w16[name][:, dci, j * 128 : (j + 1) * 128],
                        rhs=x16[:, dci, qt * 512 : (qt + 1) * 512],
                        start=(dci == 0),
                        stop=(dci == 7),
                    )
                if name == "q":
                    nc.vector.tensor_tensor(
                        QT16F[qt][:, j, :],
                        ps[:],
                        b_sb[:, j, None].to_broadcast((128, 512)),
                        mybir.AluOpType.add,
                    )
                else:
                    for half in range(2):
                        h = 2 * j + half
                        hp = 64 * half
                        nc.vector.tensor_tensor(
                            KTz16F[hp : hp + 64, 4 * qt : 4 * qt + 4, h, :],
                            ps[hp : hp + 64, :].rearrange("p (a b) -> p a b", a=4),
                            b_sb[hp : hp + 64, j, None, None].to_broadcast(
                                (64, 4, 128)
                            ),
                            mybir.AluOpType.add,
                        )

            def proj_v(kt):
                ps = scratch.tile([128, 512], F32, tag="ps")
                for dci in range(8):
                    nc.tensor.matmul(
                        ps[:, 0:C],
                        lhsT=x16[:, dci, kt * 128 : (kt + 1) * 128],
                        rhs=w16["v"][:, dci, :],
                        start=(dci == 0),
                        stop=False,
                    )
                nc.tensor.matmul(
                    ps[:, 0:C], lhsT=ones_r[:], rhs=bv_r[:], start=False, stop=True
                )
                vst = cp.tile([128, C], F8, tag="vst", bufs=2)
                nc.vector.tensor_copy(vst[:], ps[:, 0:C])
                nc.gpsimd.tensor_copy(
                    Vp8[:, kt // 2, :, kt % 2, 64 : 64 + DK],
                    vst[:].rearrange("p (a b) -> p a b", a=4),
                )
                if kt == 0:
                    nc.vector.tensor_copy(
                        Vp16[:, :, 64 : 64 + DK],
                        ps[:, 0:C].rearrange("p (a b) -> p a b", a=4),
                    )

            # upfront: only what the very first score tile needs
            proj_qk("q", 0, 0)
            proj_qk("k", 0, 0)

            # deferred groups woven into attention: (need_qt, need_h, fn)
            deferred = [(0, 0, (lambda k: lambda: proj_v(k))(kt)) for kt in range(4)]
            deferred += [
                (0, 2, lambda: proj_qk("k", 1, 0)),
                (0, 2, lambda: proj_qk("q", 1, 0)),
            ]
            for qt in range(1, 4):
                deferred.append((qt, 0, (lambda q: lambda: proj_qk("q", 0, q))(qt)))
                deferred.append((qt, 0, (lambda q: lambda: proj_qk("k", 0, q))(qt)))
                deferred.append((qt, 2, (lambda q: lambda: proj_qk("q", 1, q))(qt)))
                deferred.append((qt, 2, (lambda q: lambda: proj_qk("k", 1, q))(qt)))
                for kt in range(4 * qt, 4 * qt + 4):
                    deferred.append((qt, 0, (lambda k: lambda: proj_v(k))(kt)))

            def drain(qt, h, count=None):
                # FIFO order matches need order, so popping early is safe
                if count is not None:
                    for _ in range(count):
                        if not deferred:
                            break
                        deferred.pop(0)[2]()
                while deferred and deferred[0][0:2] <= (qt, h):
                    deferred.pop(0)[2]()

            # ======== phase C: attention + woven output projection ========
            def normalize(h, qt, pso, ncols=512, ot=None):
                hj, hp = h // 2, 64 * (h % 2)
                rcp = cp.tile([1, 512], F32, tag="rcp", bufs=2)
                rec = nc.vector.reciprocal_approx_fast(
                    rcp[:, 0:ncols], pso[0:1, 0:ncols]
                )
                tc.chain_iter_dep("nrm", rec.ins)
                rb = cp.tile([64, 512], F32, tag="rb", bufs=2)
                pb = nc.gpsimd.partition_broadcast(
                    rb[:, 0:ncols], rcp[:, 0:ncols], channels=64
                )
                tc.chain_iter_dep("nrm", pb.ins)
                dst = OTs[qt][hp : hp + 64, hj, 0:ncols] if ot is None else ot
                ml = nc.vector.tensor_tensor(
                    dst,
                    pso[64:128, 0:ncols],
                    rb[:, 0:ncols],
                    mybir.AluOpType.mult,
                )
                tc.chain_iter_dep("nrm", ml.ins)

            def outproj_piece(qt, sub, e):
                q0 = qt * 512 + sub * 128
                psy = scratch.tile([128, 512], F32, tag="ps")
                for cj in range(2):
                    nc.tensor.matmul(
                        psy[:],
                        lhsT=OTs[qt][:, cj, sub * 128 : (sub + 1) * 128],
                        rhs=Wo16[:, cj, e * 512 : (e + 1) * 512],
                        start=(cj == 0),
                        stop=(cj == 1),
                    )
                y_sb = cp.tile([128, 512], F32, tag="y", bufs=3)
                nc.vector.tensor_copy(y_sb[:], psy[:])
                nc.sync.dma_start(
                    out_d[q0 : q0 + 64, e * 512 : (e + 1) * 512], y_sb[0:64]
                )
                nc.sync.dma_start(
                    out_d[q0 + 64 : q0 + 128, e * 512 : (e + 1) * 512],
                    y_sb[64:128],
                )

            def outproj(qt):
                for sub in range(4):
                    for e in range(2):
                        outproj_piece(qt, sub, e)

            def precise_rows():
                # f16 p/v recompute of q rows 0:128, overwrites OT[0][:, :, 0:128]
                for h in range(GH):
                    hj, hp = h // 2, 64 * (h % 2)
                    ps16 = scratch.tile([128, 512], F32, tag="ps")
                    nc.tensor.matmul(
                        ps16[:, 0:128],
                        lhsT=KTz16F[:, 0, h, :],
                        rhs=QT16F[0][:, hj, 0:128],
                        start=True,
                        stop=True,
                    )
                    p16 = cp.tile([128, 128], F16, tag="p16", bufs=2)
                    nc.scalar.activation(
                        p16[:],
                        ps16[:, 0:128],
                        mybir.ActivationFunctionType.Exp,
                        scale=EXP_SCALE,
                        bias=ebias[:],
                    )
                    nc.gpsimd.affine_select(
                        out=p16[:],
                        in_=p16[:],
                        pattern=[[1, 128]],
                        compare_op=mybir.AluOpType.is_ge,
                        fill=0.0,
                        base=0,
                        channel_multiplier=-1,
                    )
                    pso16 = scratch.tile([128, 512], F32, tag="ps")
                    nc.tensor.matmul(
                        pso16[:128, 0:128],
                        lhsT=Vp16[:, h, :],
                        rhs=p16[:],
                        start=True,
                        stop=True,
                    )
                    normalize(
                        h, 0, pso16, ncols=128, ot=OTs[0][hp : hp + 64, hj, 0:128]
                    )

            def pv(p8, m, h, pso, npairs, qt):
                qlo = max(0, (2 * m - 4 * qt) * 128)
                nc.tensor.matmul(
                    pso[:VP, qlo:512],
                    lhsT=Vp8[:, m, h, :, :],
                    rhs=p8[:, :, qlo:512],
                    start=(m == 0),
                    stop=(m == npairs - 1),
                    perf_mode=DR,
                )

            carry = None
            pending = None
            for qt in range(QT_TILES):
                npairs = 2 * (qt + 1)
                for h in range(GH):
                    hj = h // 2
                    drain(qt, h)
                    pso = ppo.tile([VP, 512], F32, tag="pso", name=f"pso{qt}{h}")
                    for m in range(npairs):
                        pss = pp2.tile([128, 2, 512], F32, tag="pss")
                        for i in range(2):
                            kt = 2 * m + i
                            qlo = max(0, (kt - 4 * qt) * 128)
                            nc.tensor.matmul(
                                pss[:, i, qlo:512],
                                lhsT=KTz16F[:, kt, h, :],
                                rhs=QT16F[qt][:, hj, qlo:512],
                                start=True,
                                stop=True,
                            )
                        p8 = cp.tile([128, 2, 512], F8P, tag="p8", bufs=6)
                        qlo_p = max(0, (2 * m - 4 * qt) * 128)
                        nc.scalar.activation(
                            p8[:, :, qlo_p:512],
                            pss[:, :, qlo_p:512],
                            mybir.ActivationFunctionType.Exp,
                            scale=EXP_SCALE,
                            bias=ebias[:],
                        )
                        d_even = 2 * m - 4 * qt
                        if d_even >= 0:
                            # boundary-only masks; fully-masked cols left of
                            # the pair are excluded via the PV column trim
                            c0 = d_even * 128
                            nc.gpsimd.affine_select(
                                out=p8[:, 0, c0 : c0 + 128],
                                in_=p8[:, 0, c0 : c0 + 128],
                                pattern=[[1, 128]],
                                compare_op=mybir.AluOpType.is_ge,
                                fill=0.0,
                                base=0,
                                channel_multiplier=-1,
                            )
                            nc.gpsimd.affine_select(
                                out=p8[:, 1, c0 : c0 + 256],
                                in_=p8[:, 1, c0 : c0 + 256],
                                pattern=[[1, 256]],
                                compare_op=mybir.AluOpType.is_ge,
                                fill=0.0,
                                base=-128,
                                channel_multiplier=-1,
                            )
                        if carry is not None:
                            pv(**carry)
                            carry = None
                        if m == 0:
                            if pending is not None:
                                normalize(*pending)
                                pending = None
                            if h == 0 and qt > 0:
                                if qt == 1:
                                    precise_rows()
                                for _sub in range(4):
                                    for _e in range(2):
                                        deferred.append(
                                            (
                                                qt,
                                                9,
                                                (
                                                    lambda a, b, c: lambda: outproj_piece(
                                                        a, b, c
                                                    )
                                                )(qt - 1, _sub, _e),
                                            )
                                        )
                        carry = dict(p8=p8, m=m, h=h, pso=pso, npairs=npairs, qt=qt)
                        drain(qt, h, count=2 if qt == 0 else 1)
                    pending = (h, qt, pso)
            pv(**carry)
            normalize(*pending)
            while deferred:
                deferred.pop(0)[2]()
            outproj(QT_TILES - 1)

    nc.compile()
    return nc


_NC_CACHE = None


def _get_program():
    global _NC_CACHE
    if _NC_CACHE is None:
        _NC_CACHE = _build_program()
    return _NC_CACHE


def _chunked(mat_t, nch, cols):
    """[D, cols] -> [128, nch, cols] d-chunked layout."""
    return np.ascontiguousarray(mat_t.reshape(nch, 128, cols).transpose(1, 0, 2))


def _make_in_maps(x, Wq, bq, Wk, bk, Wv, bv, Wo, bo):
    f32 = lambda v: np.asarray(v, dtype=np.float32)
    x = f32(x)
    in_maps = []
    x16_by_b = {}
    for b in range(B):
        xt = np.ascontiguousarray(x[b].T).astype(BF)  # [D, L]
        x16_by_b[b] = _chunked(xt, 8, L)
    for core in range(NCORES):
        b, g = divmod(core, 4)
        s = slice(g * C, (g + 1) * C)
        im = {
            "x16": x16_by_b[b],
            "bq": np.ascontiguousarray(WS * f32(bq)[s]),
            "bk": np.ascontiguousarray(WS * f32(bk)[s]),
            "bv": np.ascontiguousarray(WS * f32(bv)[s]),
        }
        for nm, W in (("q", Wq), ("k", Wk), ("v", Wv)):
            wt = np.ascontiguousarray((WS * f32(W)[s, :]).T).astype(BF)  # [D, C]
            im[f"w16{nm}"] = _chunked(wt, 8, C)
        wo_t = np.ascontiguousarray((f32(Wo)[:, s] / WS).T)  # [C, D]
        im["wo16"] = np.ascontiguousarray(
            wo_t.astype(BF).reshape(2, 128, D).transpose(1, 0, 2)
        )
        in_maps.append(im)
    return in_maps


def _run(in_maps, trace=False, **kw):
    nc = _get_program()
    return run_bass_kernel_spmd(nc, in_maps, list(range(NCORES)), trace=trace, **kw)


def kernel(x, Wq, bq, Wk, bk, Wv, bv, Wo, bo, _trace=False, _trace_out=None, _tmpdir=None):
    in_maps = _make_in_maps(x, Wq, bq, Wk, bk, Wv, bv, Wo, bo)
    res = _run(in_maps, trace=_trace, tmpdir=_tmpdir)
    if _trace_out is not None:
        _trace_out.append(res)
    bo = np.asarray(bo, dtype=np.float32)
    out = np.empty((B, L, D), dtype=np.float32)
    for b in range(B):
        acc = res.results[4 * b]["out"].astype(np.float32)
        for g in range(1, 4):
            acc = acc + res.results[4 * b + g]["out"]
        out[b] = acc + bo[None, :]
    return out
